# revision 34
# baseline (speedup 1.0000x reference)
"""Domain-adaptive attention on 8 Trainium2 NeuronCores.

Sharding: (batch, head-group) - cores 0-3 take batch 0, cores 4-7 batch 1;
each core owns 3 of the 12 heads. All matmul operands are bf16 (PSUM
accumulation stays f32); x is transposed and weights are sliced/cast on
the host, so the device does no transposes at all:

  xT [768, S] bf16     host-transposed, DMA'd straight into SBUF
  qT/kT = Wqk.T @ xT   (PE; per-head 64-row slabs, 1024-col moving)
  v natural            = xT-block.T @ Wv  (PE, out = [128 tokens, 192])
  scoresT_j = kT_j.T @ qT  ([128 keys x 1024 queries] psum tiles)
  aT_j = exp(dscale/sqrt(D) * scoresT_j)  (ACT, bf16 out)
  numT/den = [v | 1].T @ aT_j accumulated (PE, M=65: row 64 = denom)
  pIn = numT * (1/den)  (DVE mult; recip broadcast via PE ones-matmul)
  projT = Wp.T-tiles @ pIn  (PE) -> partial [768, S] f32 -> HBM

Host: computes dscale, folds bv into bp (softmax rows sum to 1 so
attn@(v+bv) = attn@v + bv), drops bk (adds a per-query constant to
scores, softmax-invariant), sums the 4 partial projections per batch.
Softmax skips max-subtraction: logits are bounded so exp fits f32.
"""

import numpy as np
import ml_dtypes

import concourse.bass as bass
import concourse.mybir as mybir
import concourse.tile as tile
from concourse import bacc
from concourse.bass import ds
from concourse.bass_utils import run_bass_kernel_spmd

F32 = mybir.dt.float32
F32R = mybir.dt.float32r
BF16 = mybir.dt.bfloat16
FP16 = mybir.dt.float16
AF = mybir.ActivationFunctionType
F16 = np.float16

B, S, E, H, D = 2, 2048, 768, 12, 64
HPC = 3          # heads per core
CPB = 4          # cores per batch
NCORES = 8
KT = E // 128    # 6 contraction tiles for the projections
SJ = S // 128    # 16 key tiles
MOV = 1024       # moving-dim chunk (bf16 allows 1024)
NB = S // MOV

# per-head (row-offset, plane) in qT/kT
HEAD_RP = [(0, 0), (64, 0), (0, 1)]

TRACE = False
LAST_RESULT = None


def build_nc():
    nc = bacc.Bacc(None, target_bir_lowering=False)

    xks = [
        nc.dram_tensor(f"xk{ko}", [128, S], FP16, kind="ExternalInput")
        for ko in range(KT)
    ]
    wqk = nc.dram_tensor("wqk", [E, 6 * D], FP16, kind="ExternalInput")
    wv = nc.dram_tensor("wv", [E, HPC * D], FP16, kind="ExternalInput")
    wp = nc.dram_tensor("wp", [HPC * D, E], FP16, kind="ExternalInput")
    scl = nc.dram_tensor("scl", [128, HPC], F32, kind="ExternalInput")
    bqc = nc.dram_tensor("bqc", [128, 2], F32, kind="ExternalInput")
    outp = nc.dram_tensor("outp", [E, S], F32, kind="ExternalOutput")

    with tile.TileContext(nc) as tc:
        with (
            tc.tile_pool(name="persist", bufs=1) as pp,
            tc.tile_pool(name="at", bufs=5) as atp,
            tc.tile_pool(name="norm", bufs=2) as nrp,
            tc.tile_pool(name="prout", bufs=4) as prp,
            tc.tile_pool(name="ps", bufs=2, space="PSUM") as psp,
            tc.tile_pool(name="po", bufs=1, space="PSUM") as pop,
            tc.tile_pool(name="vps", bufs=2, space="PSUM") as vpp,
        ):
            # ---- persistent SBUF tiles ----
            xT = pp.tile([128, KT, S], FP16, tag="xT")
            wqk_sb = pp.tile([128, KT, 6 * D], FP16, tag="wqk_sb")
            wv_sb = pp.tile([128, KT, HPC * D], FP16, tag="wv_sb")
            wp_sb = pp.tile([128, 2, E], FP16, tag="wp_sb")
            scl_sb = pp.tile([128, HPC], F32, tag="scl_sb")
            bq_sb = pp.tile([128, 2], F32, tag="bq_sb")
            qT = pp.tile([128, 2, S], FP16, tag="qT")
            kT = pp.tile([128, 2, S], FP16, tag="kT")
            # v natural per key tile: 3 heads x (64 v-cols + ones col + pad)
            v_sb = pp.tile([128, SJ, HPC, 128], BF16, tag="v_sb")
            pIn = pp.tile([128, 2, S], FP16, tag="pIn")
            ones64 = pp.tile([1, 64], F32R, tag="ones64")

            # ---- loads ----
            nc.sync.dma_start(out=wqk_sb, in_=wqk.rearrange("(ko p) m -> p ko m", p=128))
            for ko in range(KT):
                nc.sync.dma_start(out=xT[:, ko, :], in_=xks[ko][:, :])
            nc.sync.dma_start(out=wv_sb, in_=wv.rearrange("(ko p) m -> p ko m", p=128))
            nc.sync.dma_start(out=wp_sb[:, 0, :], in_=wp[0:128, :])
            nc.sync.dma_start(out=wp_sb[0:64, 1, :], in_=wp[128:192, :])
            nc.sync.dma_start(out=scl_sb, in_=scl[:, :])
            nc.sync.dma_start(out=bq_sb, in_=bqc[:, :])
            ones_f = pp.tile([1, 64], F32, tag="ones_f")
            nc.vector.memset(ones_f, 1.0)
            nc.vector.tensor_copy(ones64, ones_f)
            onesc = pp.tile([128, 1], F32, tag="onesc")
            nc.vector.memset(onesc, 1.0)
            # zero the tail columns of every v stationary tile once; the
            # denominator ones-column and v block get overwritten per tile
            nc.vector.memset(v_sb[:, :, :, 64:128], 0.0)

            # ---- q/k projections ----
            # wqk cols: [q0|q1] [k0|k1] [q2|k2] as three 128-col chain groups
            def qk_chain(t, hf):
                # hf indexes a 512-token half; q rows get (x@Wq + bq) * dscale
                pst = psp.tile([128, MOV], F32, tag="ps", name=f"qk_{t}_{hf}")
                ps = pst[:, 0:512]
                for ko in range(KT):
                    nc.tensor.matmul(
                        ps,
                        lhsT=wqk_sb[:, ko, ds(t * 128, 128)],
                        rhs=xT[:, ko, ds(hf * 512, 512)],
                        start=(ko == 0),
                        stop=(ko == KT - 1),
                    )
                if t == 0:
                    nc.vector.tensor_scalar_add(
                        out=qT[:, 0, ds(hf * 512, 512)],
                        in0=ps,
                        scalar1=bq_sb[:, 0:1],
                    )
                elif t == 1:
                    nc.vector.tensor_copy(kT[:, 0, ds(hf * 512, 512)], ps)
                else:
                    nc.vector.tensor_scalar_add(
                        out=qT[0:64, 1, ds(hf * 512, 512)],
                        in0=ps[0:64, :],
                        scalar1=bq_sb[0:64, 1:2],
                    )
                    nc.vector.tensor_copy(
                        kT[0:64, 1, ds(hf * 512, 512)], ps[64:128, :]
                    )

            # only k-keys 0-1023 and q-half0 are needed before the first
            # scores; the other four chains ride inside h0-half0
            for t in range(2):
                for hf in range(2):
                    qk_chain(t, hf)

            # ---- v projection, natural orientation ----
            def v_tile(j):
                psv = vpp.tile([128, HPC, D], F32, tag="vps", name=f"v_{j}")
                for ko in range(KT):
                    nc.tensor.matmul(
                        psv[:, :, :],
                        lhsT=xT[:, ko, ds(j * 128, 128)],
                        rhs=wv_sb[:, ko, :],
                        start=(ko == 0),
                        stop=(ko == KT - 1),
                    )
                # scatter 3 heads' 64-col blocks into 66-col groups (bf16)
                nc.vector.tensor_copy(v_sb[:, j, :, 0:64], psv[:, :, :])
                for h in range(HPC):
                    nc.vector.tensor_copy(v_sb[:, j, h, 64:65], onesc)

            def warm_mm():
                wps = vpp.tile([128, HPC, D], F32, tag="vps", name="warm")
                nc.tensor.matmul(
                    wps[0:64, 0, 0:64],
                    lhsT=ones64,
                    rhs=ones64,
                    start=True,
                    stop=True,
                )

            # ---- attention ----
            def normalize(h, half, po):
                ro, pl = HEAD_RP[h]
                den = nrp.tile([1, MOV], F32, tag="den")
                nc.vector.tensor_copy(den, po[64:65, :])
                rc = nrp.tile([1, MOV], F32, tag="rc")
                nc.vector.reciprocal_approx_fast(out=rc, in_=den)
                pn = nrp.tile([64, MOV], F32, tag="pn")
                nc.vector.tensor_copy(pn, po[0:64, :])
                rcr = nrp.tile([1, MOV], F32R, tag="rcr")
                nc.vector.tensor_copy(rcr, rc)
                pb = pop.tile([64, MOV], F32, tag="po", name=f"pb_{h}_{half}")
                for q2 in range(MOV // 512):
                    nc.tensor.matmul(
                        pb[:, ds(q2 * 512, 512)],
                        lhsT=ones64,
                        rhs=rcr[:, ds(q2 * 512, 512)],
                        start=True,
                        stop=True,
                    )
                nc.vector.tensor_mul(
                    pIn[ro : ro + 64, pl, ds(half * MOV, MOV)],
                    pn,
                    pb,
                )

            def attend(h, half, extra=()):
                extra = list(extra)
                ro, pl = HEAD_RP[h]
                po = pop.tile([128, MOV], F32, tag="po", name=f"po_{h}_{half}")
                for j in range(SJ):
                    ps = psp.tile([128, MOV], F32, tag="ps", name=f"sc_{h}_{half}_{j}")
                    for q2 in range(MOV // 512):
                        nc.tensor.matmul(
                            ps[:, ds(q2 * 512, 512)],
                            lhsT=kT[ro : ro + 64, pl, ds(j * 128, 128)],
                            rhs=qT[ro : ro + 64, pl, ds(half * MOV + q2 * 512, 512)],
                            start=True,
                            stop=True,
                        )
                    at = atp.tile([128, MOV], BF16, tag="at", name=f"at_{h}_{half}_{j}")
                    nc.scalar.activation(
                        out=at, in_=ps, func=AF.Exp, scale=scl_sb[:, h : h + 1]
                    )
                    if extra:
                        extra.pop(0)()
                    for q2 in range(MOV // 512):
                        nc.tensor.matmul(
                            po[:, ds(q2 * 512, 512)],
                            lhsT=v_sb[:, j, h, :],
                            rhs=at[:, ds(q2 * 512, 512)],
                            start=(j == 0),
                            stop=(j == SJ - 1),
                        )
                normalize(h, half, po)

            def proj_piece(mt, nb):
                pr = psp.tile([128, MOV], F32, tag="ps", name=f"pj_{mt}_{nb}")
                for q2 in range(MOV // 512):
                    nc.tensor.matmul(
                        pr[:, ds(q2 * 512, 512)],
                        lhsT=wp_sb[:, 0, ds(mt * 128, 128)],
                        rhs=pIn[:, 0, ds(nb * MOV + q2 * 512, 512)],
                        start=True,
                        stop=False,
                    )
                    nc.tensor.matmul(
                        pr[:, ds(q2 * 512, 512)],
                        lhsT=wp_sb[0:64, 1, ds(mt * 128, 128)],
                        rhs=pIn[0:64, 1, ds(nb * MOV + q2 * 512, 512)],
                        start=False,
                        stop=True,
                    )
                prs = prp.tile([128, MOV], F32, tag="prs", name=f"pjs_{mt}_{nb}")
                nc.vector.tensor_copy(prs, pr)
                nc.sync.dma_start(
                    out=outp[ds(mt * 128, 128), ds(nb * MOV, MOV)], in_=prs
                )

            # v tiles 0-1 up front; the rest + the T2 q/k chains + proj
            # ride inside the attention halves as PE filler. Filler-free
            # halves get tiny warm-keeper matmuls so HAM never re-throttles.
            for j in range(4):
                v_tile(j)
            vt = lambda jj: (lambda: v_tile(jj))
            pj = lambda m, nb: (lambda: proj_piece(m, nb))
            # all v chains must issue within h0-half0 (PV j reads v_sb[j]);
            # deferred k/q chains slot in at iters that respect their first use
            # (kT hf2 by j=8, hf3 by j=12, q half1 before h0-half1 starts)
            qkd = lambda t, f: (lambda: qk_chain(t, f))
            h00 = [
                vt(4), vt(5), qkd(1, 2), vt(6),
                vt(7), qkd(1, 3), vt(8), vt(9),
                qkd(0, 2), vt(10), vt(11), qkd(0, 3),
                vt(12), vt(13), vt(14), vt(15),
            ]
            attend(0, 0, extra=h00)
            attend(0, 1, extra=[qkd(2, hf) for hf in range(4)])
            attend(1, 0)
            attend(1, 1)
            attend(2, 0)
            attend(2, 1, extra=[pj(mt, 0) for mt in range(6)])
            for mt in range(6):
                proj_piece(mt, 1)

    nc.compile()
    return nc


_NC = None


def _get_nc():
    global _NC
    if _NC is None:
        _NC = build_nc()
    return _NC


def ds2(ko):
    return slice(ko * 128, (ko + 1) * 128)


def make_in_maps(x, domain_embedding, Wq, bq, Wk, bk, Wv, bv, Wd, bd, Wp, bp):
    f = lambda a: np.ascontiguousarray(np.asarray(a, dtype=np.float32))
    x, domain_embedding = f(x), f(domain_embedding)
    Wq, Wk, Wv, Wp, Wd = f(Wq), f(Wk), f(Wv), f(Wp), f(Wd)
    bq, bk, bv, bd = f(bq), f(bk), f(bv), f(bd)

    dscale = domain_embedding @ Wd + bd  # [B, H]
    bf = lambda a: np.ascontiguousarray(a.astype(F16))
    in_maps = []
    for c in range(NCORES):
        b, h0 = c // CPB, HPC * (c % CPB)
        cols = slice(D * h0, D * h0 + HPC * D)
        xb_T = np.ascontiguousarray(x[b].T)  # [E, S]
        q = Wq[:, cols]  # [E, 192]
        k = Wk[:, cols]
        # wqk cols: q0 q1 | k0 k1 | q2 k2
        wqk_np = np.concatenate(
            [q[:, 0:128], k[:, 0:128], q[:, 128:192], k[:, 128:192]], axis=1
        )
        bq_np = np.zeros((128, 2), np.float32)
        bq_np[:, 0] = bq[cols][0:128]
        bq_np[0:64, 1] = bq[cols][128:192]
        scl_np = np.tile(
            (dscale[b, h0 : h0 + HPC] * (1.0 / np.sqrt(D))).astype(np.float32)[None, :],
            (128, 1),
        )
        im = {f"xk{ko}": bf(xb_T[ds2(ko)]) for ko in range(KT)}
        in_maps.append(
            {
                **im,
                "wqk": bf(wqk_np),
                "wv": bf(Wv[:, cols]),
                "wp": bf(Wp[cols, :]),
                "scl": np.ascontiguousarray(scl_np),
                "bqc": bq_np,
            }
        )
    return in_maps


def kernel(x, domain_embedding, Wq, bq, Wk, bk, Wv, bv, Wd, bd, Wp, bp):
    global LAST_RESULT
    in_maps = make_in_maps(
        x, domain_embedding, Wq, bq, Wk, bk, Wv, bv, Wd, bd, Wp, bp
    )
    res = run_bass_kernel_spmd(
        _get_nc(), in_maps, core_ids=list(range(NCORES)), trace=TRACE
    )
    LAST_RESULT = res
    # bv passes straight through attention (softmax rows sum to 1) and
    # then through the projection; bk only shifts scores by a per-query
    # constant (softmax-invariant).
    bp_eff = (
        np.asarray(bp, dtype=np.float32)
        + np.asarray(bv, dtype=np.float32) @ np.asarray(Wp, dtype=np.float32)
    )
    out = np.empty((B, S, E), np.float32)
    for b in range(B):
        acc = res.results[CPB * b]["outp"].copy()
        for c in range(CPB * b + 1, CPB * (b + 1)):
            acc += res.results[c]["outp"]
        out[b] = acc.T + bp_eff[None, :]
    return out
